# revision 12
# baseline (speedup 1.0000x reference)
"""Trainium2 Bass kernel for nn_DSC_PO_29721173688901.

Math (reference): u = -K y_obs + first(y_nat) + second(y_nat, hist) + bias
where y_nat = y_obs - effect, effect[b] = sum_{t=0..511} C A^t B u_{b,t}.

Everything is linear, so u = Qall y_obs + sum_{k>=1} D_k hist_k + bias
+ Pn R with R = sum_t A^t B u_t, Qall = -K + W0 + D_0, Pn = -(W0+D0) C.
All terms except Pn R are O(MC*N*B) input prep, folded on host; the
device computes only R's batched matmul chain and z_r.

Strided Horner decomposition with stride 64 across 8 cores:
  t = rho + 64 q,  rho = r + 8 c  (r = core 0..7, c = chain 0..7, q = 0..7)
  H_rho = sum_q (A^64)^q v_{rho+64q}   (Horner, 7 steps, folded v-adds)
  z_r   = sum_c (Pn A^{8c}) H_{r+8c}   (16x64)
  u     = sum_r z_r + host consts      (host gather/sum of 8 core outputs)
The per-core A^r factor rides in a one-hot-extended U (rows 16r:16r+16
hold the controls) against B2 = [B_0..B_7]^T, so the program is
rank-uniform.  The squaring ladder runs in bf16 with PE is_transpose
passes instead of transpose products; only its last output (A^64)^T is
cast to fp8e4m3 (x16 to dodge denormals).  The 512-wide Horner then
runs on fp8 DoubleRow matmuls (both the A^64 terms and the zero-padded
B-term), which fully hides the 256-row weight loads.  The tiny fold
matrices w_c = (A^{8c})^T Pn^T are built in bf16 inside ladder stalls,
so the post-Horner tail is 32 narrow matmuls and one 4KB DMA out.  No
collective; the 8 per-core z_r are summed on host.
"""

import numpy as np
import ml_dtypes

import concourse.bacc as bacc
import concourse.mybir as mybir
from concourse.bass_utils import run_bass_kernel_spmd
from concourse.tile import TileContext
from concourse.masks import make_identity

N = 512
MC = 16
T = 512
BATCH = 64
N_CORES = 8
STRIDE = 64
QLEN = T // STRIDE    # 8 Horner slots per chain
NCH = STRIDE // N_CORES   # 8 chains per core
KT = N // 128         # 4 contraction tiles
BF = mybir.dt.bfloat16
F32 = mybir.dt.float32
F8 = mybir.dt.float8e4
SC = 16.0             # fp8 carry scale
DR = mybir.MatmulPerfMode.DoubleRow

_COMPILED = {}


def _build_nc():
    nc = bacc.Bacc("TRN2", target_bir_lowering=False)

    d_A = nc.dram_tensor("Amat", (128, KT, N), BF, kind="ExternalInput")
    d_AT = nc.dram_tensor("ATmat", (128, KT, N), BF, kind="ExternalInput")
    d_BT = nc.dram_tensor("BTmat", (MC, N), F8, kind="ExternalInput")
    d_Bk = nc.dram_tensor("Bkmat", (128, KT, MC), BF, kind="ExternalInput")
    d_P = nc.dram_tensor("PnT", (128, KT, MC), BF, kind="ExternalInput")
    # Uhot rows: 128 = 8 j-blocks x 16 controls (block r holds this core's u,
    # x16 fp8); cols: slot j (8) x dr-pad (2) x (chain (8) x batch (64))
    d_U = nc.dram_tensor("Ucore", (128, QLEN, 2, NCH * BATCH), F8,
                         kind="ExternalInput")
    d_out = nc.dram_tensor("uT", (MC, BATCH), F32, kind="ExternalOutput")

    with TileContext(nc) as tc:
        with tc.tile_pool(name="w", bufs=1) as wpool, \
             tc.tile_pool(name="st", bufs=1) as st_pool:

            def wtile(name, shape, dt=BF):
                return wpool.tile(shape, dt, tag=name, name=name)

            t_A = wtile("A", [128, KT, N])
            t_AT = wtile("AT", [128, KT, N])
            t_I = wtile("I", [128, 128])
            t_U = wtile("U", [128, QLEN, 2, NCH * BATCH], F8)
            # B2: DR-padded B-term weights; [:, 0, :] row-block j = (A^j B)^T
            # (x16 fp8), [:, 1, :] zeros
            t_B2 = wtile("B2", [128, 2, N], F8)
            # untransposed bf16 [B_0 | ... | B_7], k-tiled (b-chain rhs only)
            t_Ball = wtile("Ball", [128, KT, N_CORES * MC])
            # fold matrices w_c = (A^{8c})^T Pn^T, c = 0..7 (c=0 is Pn^T)
            t_w = wtile("wf", [128, KT, NCH, MC])

            # k-chunked so the first product can start mid-transfer
            for k in range(KT):
                nc.sync.dma_start(out=t_AT[:, k, :], in_=d_AT[:, k, :])
                nc.sync.dma_start(out=t_A[:, k, :], in_=d_A[:, k, :])
            nc.sync.dma_start(out=t_B2[0:MC, 0, :], in_=d_BT[:])
            nc.sync.dma_start(out=t_Ball[:, :, 0:MC], in_=d_Bk[:])
            nc.sync.dma_start(out=t_w[:, :, 0, :], in_=d_P[:])
            nc.sync.dma_start(out=t_U[:], in_=d_U[:])

            t_A2 = wtile("A2", [128, KT, N])
            t_AT2 = wtile("AT2", [128, KT, N])
            t_A4 = wtile("A4", [128, KT, N])
            t_AT4 = wtile("AT4", [128, KT, N])
            t_A8 = wtile("A8", [128, KT, N])
            t_AT8 = wtile("AT8", [128, KT, N])
            t_A16 = wtile("A16", [128, KT, N])
            t_AT16 = wtile("AT16", [128, KT, N])
            t_A32 = wtile("A32", [128, KT, N])
            t_AT32 = wtile("AT32", [128, KT, N])
            t_A64 = wtile("A64", [128, KT, N], F8)   # (A^64)^T, x16 fp8

            # identity built on-device (no DMA dep) for PE transposes and
            # for clock-ramp warmup matmuls during the input DMA window
            make_identity(nc, t_I[:])
            # zero the DR pad rows of the B-term weights
            nc.vector.memset(t_B2[:, 1, :], 0.0)

            # ---- phase 1: bf16 squaring ladder + transposes + B-chain ----
            with tc.tile_pool(name="psq", bufs=1, space="PSUM") as psq_pool:

                def product(out_t, lhsT_t, rhs_t, pname, f8=False):
                    # k-outer with 4 concurrent PSUM groups: consumes the
                    # previous transpose pass's tiles in emission order, and
                    # lets the first matmuls start on partial inputs.
                    pss = [psq_pool.tile([128, N], F32, tag="psq",
                                         bufs=4, name=f"psq_{pname}_{m}")
                           for m in range(KT)]
                    for k in range(KT):
                        for m in range(KT):
                            nc.tensor.matmul(
                                pss[m][:],
                                lhsT_t[:, k, 128 * m:128 * (m + 1)],
                                rhs_t[:, k, :],
                                start=(k == 0), stop=(k == KT - 1),
                            )
                    for m in range(KT):
                        # split across both engines: halves eviction latency
                        # and unblocks per-128-col transpose consumers early
                        if f8:
                            nc.vector.tensor_scalar_mul(
                                out_t[:, m, 0:256], pss[m][:, 0:256], SC)
                            nc.scalar.activation(
                                out_t[:, m, 256:N], pss[m][:, 256:N],
                                mybir.ActivationFunctionType.Copy, scale=SC)
                        else:
                            nc.vector.tensor_copy(out=out_t[:, m, 0:256],
                                                  in_=pss[m][:, 0:256])
                            nc.scalar.activation(
                                out_t[:, m, 256:N], pss[m][:, 256:N],
                                mybir.ActivationFunctionType.Copy)

                # two alternating transpose banks so consecutive
                # is_transpose ops pipeline (same-bank matmuls serialize)
                ps_trA = psq_pool.tile([128, 8, 128], BF, tag="ptrA", bufs=1,
                                       name="ps_trA")
                ps_trB = psq_pool.tile([128, 8, 128], BF, tag="ptrB", bufs=1,
                                       name="ps_trB")

                # PE clock-ramp warmup: dummy ident transposes that only
                # depend on the on-device identity, filling the DMA window
                for wi in range(40):
                    sl = (ps_trA if wi % 2 == 0
                          else ps_trB)[:, (wi // 2) % 8, :]
                    nc.tensor.transpose(sl, t_I[:], t_I[:])

                def transpose_mat(out_t, in_t, pname):
                    # out = in^T via PE is_transpose; one 128x128 tile per
                    # instruction, bf16 PSUM pass-through.
                    idx = 0
                    for o in range(KT):
                        for i in range(KT):
                            sl = (ps_trA if idx % 2 == 0
                                  else ps_trB)[:, (idx // 2) % 8, :]
                            nc.tensor.transpose(
                                sl, in_t[:, i, 128 * o:128 * (o + 1)],
                                t_I[:])
                            if idx % 2 == 0:
                                nc.vector.tensor_copy(
                                    out=out_t[:, o, 128 * i:128 * (i + 1)],
                                    in_=sl)
                            else:
                                nc.scalar.activation(
                                    out_t[:, o, 128 * i:128 * (i + 1)],
                                    sl, mybir.ActivationFunctionType.Copy)
                            idx += 1

                def b_batch(nb, lhsT_t, pname):
                    # untransposed: [B_nb..B_{2nb-1}] = A^nb [B_0..B_{nb-1}]
                    # (lhsT_t = (A^nb)^T); also transposed rows of B2 (fp8).
                    w = MC * nb
                    for m in range(KT):
                        psf = psq_pool.tile([128, NCH * MC], F32, tag="psbu",
                                            bufs=2, name=f"psbu_{pname}_{m}")
                        ps = psf[:, 0:w]
                        for k in range(KT):
                            nc.tensor.matmul(
                                ps,
                                lhsT_t[:, k, 128 * m:128 * (m + 1)],
                                t_Ball[:, k, 0:w],
                                start=(k == 0), stop=(k == KT - 1),
                            )
                        nc.vector.tensor_copy(
                            out=t_Ball[:, m, w:2 * w], in_=ps)
                    # transposed: [B_nb^T; ...] = Ball[:, :w]^T (A^nb)^T
                    psf = psq_pool.tile([128, N], F32, tag="psq", bufs=4,
                                        name=f"psbt_{pname}")
                    ps = psf[0:w, :]
                    for k in range(KT):
                        nc.tensor.matmul(
                            ps,
                            t_Ball[:, k, 0:w],
                            lhsT_t[:, k, :],
                            start=(k == 0), stop=(k == KT - 1),
                        )
                    if w % 32 == 0:
                        nc.vector.tensor_scalar_mul(
                            t_B2[w:2 * w, 0, :], ps, SC)
                    else:
                        sc = st_pool.tile([w, N], F8, tag="bt_scratch",
                                          bufs=2, name=f"btsc_{pname}")
                        nc.vector.tensor_scalar_mul(sc[:], ps, SC)
                        nc.sync.dma_start(out=t_B2[w:2 * w, 0, :], in_=sc[:])

                def w_fold(c_lo, c_hi, lhsT_t, pname):
                    # t_w[:, :, c_lo+cc] = lhsT_t^T @ t_w[:, :, cc]  (bf16)
                    wd = (c_hi - c_lo) * MC
                    for m in range(KT):
                        ps = psq_pool.tile([128, NCH * MC], F32, tag="psbu",
                                           bufs=2, name=f"psw_{pname}_{m}")
                        for k in range(KT):
                            nc.tensor.matmul(
                                ps[:, 0:wd],
                                lhsT_t[:, k, 128 * m:128 * (m + 1)],
                                t_w[:, k, 0:c_hi - c_lo, :],
                                start=(k == 0), stop=(k == KT - 1),
                            )
                        nc.vector.tensor_copy(
                            out=t_w[:, m, c_lo:c_hi, :], in_=ps[:, 0:wd])

                product(t_A2, t_AT, t_A, "A2")
                b_batch(1, t_AT, "b1")
                transpose_mat(t_AT2, t_A2, "AT2")
                product(t_A4, t_AT2, t_A2, "A4")
                b_batch(2, t_AT2, "b2")
                transpose_mat(t_AT4, t_A4, "AT4")
                product(t_A8, t_AT4, t_A4, "A8")
                b_batch(4, t_AT4, "b4")
                transpose_mat(t_AT8, t_A8, "AT8")
                product(t_A16, t_AT8, t_A8, "A16")
                w_fold(1, 2, t_A8, "w1")          # w_1 = A8^T Pn^T
                transpose_mat(t_AT16, t_A16, "AT16")
                product(t_A32, t_AT16, t_A16, "A32")
                w_fold(2, 4, t_A16, "w23")        # [w_2 w_3] = A16^T [c0 c1]
                transpose_mat(t_AT32, t_A32, "AT32")
                w_fold(4, 8, t_A32, "w4567")      # [w_4..w_7] = A32^T [c0..3]
                product(t_A64, t_A32, t_AT32, "A64", f8=True)

            # ---- phase 2: fp8 DoubleRow Horner, 512-wide, v-adds folded ----
            # state: [p, k-tile, 512 = chain(8) x batch(64)], fp8 x16;
            # every eviction rescales by 1/16; final state bf16 unscaled.
            with tc.tile_pool(name="pch", bufs=1, space="PSUM") as pch_pool:

                psu = pch_pool.tile([MC, BATCH], F32, tag="psu", bufs=1,
                                    name="psu")
                W = NCH * BATCH

                def evict_h(dst, ps, parity, scale):
                    if parity % 2 == 0:
                        nc.vector.tensor_scalar_mul(dst, ps, scale)
                    else:
                        nc.scalar.activation(
                            dst, ps, mybir.ActivationFunctionType.Copy,
                            scale=scale)

                s_cur = st_pool.tile([128, KT, W], F8, tag="s",
                                     name="s_init", bufs=3)
                for m in range(KT):
                    ps = pch_pool.tile([128, W], F32, tag="pch", bufs=7,
                                       name=f"pch_0_{m}")
                    nc.tensor.matmul(
                        ps[:], t_B2[:, :, 128 * m:128 * (m + 1)],
                        t_U[:, 0, :, :], start=True, stop=True,
                        perf_mode=DR)
                    evict_h(s_cur[:, m, :], ps[:], m, 1.0 / SC)

                for j in range(1, QLEN):
                    last = (j == QLEN - 1)
                    s_new = st_pool.tile([128, KT, W], BF if last else F8,
                                         tag=("sf" if last else "s"),
                                         name=f"s_{j}", bufs=1 if last else 3)
                    for m in range(KT):
                        ps = pch_pool.tile([128, W], F32, tag="pch", bufs=7,
                                           name=f"pch_{j}_{m}")
                        # B-term first: no dep on the previous step's last
                        # eviction, keeps the PE streaming.
                        nc.tensor.matmul(
                            ps[:], t_B2[:, :, 128 * m:128 * (m + 1)],
                            t_U[:, j, :, :], start=True, stop=False,
                            perf_mode=DR)
                        for p in range(2):
                            nc.tensor.matmul(
                                ps[:],
                                t_A64[:, 2 * p:2 * p + 2,
                                      128 * m:128 * (m + 1)],
                                s_cur[:, 2 * p:2 * p + 2, :],
                                start=False, stop=(p == 1),
                                perf_mode=DR,
                            )
                        evict_h(s_new[:, m, :], ps[:], m,
                                (1.0 / (SC * SC)) if last else (1.0 / SC))
                    s_cur = s_new

                # ---- z = sum_c w_c^T G_c  (k-outer: consumes the last
                # step's evictions in order) ----
                idx = 0
                for k in range(KT):
                    for c in range(NCH):
                        nc.tensor.matmul(
                            psu[:], t_w[:, k, c, :],
                            s_cur[:, k, 64 * c:64 * (c + 1)],
                            start=(idx == 0), stop=(idx == NCH * KT - 1))
                        idx += 1

                t_u = wtile("u", [MC, BATCH], F32)
                nc.vector.tensor_copy(out=t_u[:], in_=psu[:])
                nc.sync.dma_start(out=d_out[:], in_=t_u[:])

    nc.compile()
    return nc


def _arr512(m, dtype=ml_dtypes.bfloat16):
    """(512, X) -> (128, 4, X) k-tiled partition layout."""
    x = m.shape[1]
    return np.ascontiguousarray(
        m.reshape(KT, 128, x).transpose(1, 0, 2)).astype(dtype)


def _prep_inputs(A, B, C, K, bias, M0, M_tensor, sigma_phi_m, sigma_phi_M,
                 u_hist_rev, y_nat_history, y_obs):
    bf = ml_dtypes.bfloat16
    f8 = ml_dtypes.float8_e4m3
    A = np.asarray(A, np.float32)
    C = np.asarray(C, np.float32)
    B = np.asarray(B, np.float32)
    K = np.asarray(K, np.float32)
    U = np.asarray(u_hist_rev, np.float32)[..., 0]        # (64, 512, 16)
    ynh = np.asarray(y_nat_history, np.float32)[..., 0]   # (64, 20, 512)
    yo = np.asarray(y_obs, np.float32)[..., 0]            # (64, 512)

    s_m = np.asarray(sigma_phi_m, np.float32).sum(axis=1)
    W0 = np.einsum('chn,h->cn', np.asarray(M0, np.float32), s_m)
    D = np.einsum('cijn,ik,j->ckn', np.asarray(M_tensor, np.float32),
                  np.asarray(sigma_phi_M, np.float32), s_m)   # (16, 10, 512)
    G = W0 + D[:, 0]
    Pn = -(G @ C)                                   # (16, 512)
    Qall = -K + G

    # host constants: Qall yo + sum_{k>=1} D_k hist_k + bias   -> (64, 16)
    Yk = np.stack([ynh[:, 20 - k] for k in range(1, 10)], axis=1)  # (64,9,512)
    const = (yo @ Qall.T
             + np.einsum('ckn,bkn->bc', D[:, 1:], Yk)
             + np.asarray(bias, np.float32)[:, 0][None, :])

    common = {
        "Amat": _arr512(A),
        "ATmat": _arr512(np.ascontiguousarray(A.T)),
        "BTmat": np.ascontiguousarray(B.T * SC).astype(f8),
        "Bkmat": _arr512(B),
        "PnT": _arr512(np.ascontiguousarray(Pn.T)),
    }
    in_maps = []
    for r in range(N_CORES):
        # chains rho = r + 8c; Horner slot j handles q = QLEN-1-j; controls
        # ride in one-hot row-block r so the chain picks up B_r = A^r B.
        # DR pad subtile (index 1) stays zero.
        Uc = np.zeros((QLEN, 2, NCH, 128, 64), np.float32)
        for j in range(QLEN):
            q = QLEN - 1 - j
            for c in range(NCH):
                t = (r + 8 * c) + STRIDE * q
                Uc[j, 0, c, MC * r:MC * (r + 1), :] = U[:, t, :].T * SC
        # -> rows x (slot, pad, chain, batch)
        Uhot = Uc.transpose(3, 0, 1, 2, 4).reshape(
            128, QLEN, 2, NCH * BATCH)
        m = dict(common)
        m["Ucore"] = np.ascontiguousarray(Uhot).astype(f8)
        in_maps.append(m)
    return in_maps, const


def _run(in_maps, **kwargs):
    if "nc" not in _COMPILED:
        _COMPILED["nc"] = _build_nc()
    return run_bass_kernel_spmd(
        _COMPILED["nc"], in_maps, core_ids=list(range(N_CORES)), **kwargs)


def kernel(A, B, C, K, bias, M0, M_tensor, sigma_phi_m, sigma_phi_M,
           u_hist_rev, y_nat_history, y_obs, _profile=False):
    in_maps, const = _prep_inputs(
        A, B, C, K, bias, M0, M_tensor, sigma_phi_m, sigma_phi_M,
        u_hist_rev, y_nat_history, y_obs)
    res = _run(in_maps, trace=_profile)
    # gather/unshard: the 8 cores' partial z_r sum to Pn R
    zsum = np.zeros((MC, BATCH), np.float64)
    for r in range(N_CORES):
        zsum += res.results[r]["uT"].astype(np.float64)
    u = zsum.T.astype(np.float32) + const
    out = u[..., None].astype(np.float32)      # (64, 16, 1)
    if _profile:
        return out, res
    return out
